# revision 2
# baseline (speedup 1.0000x reference)
"""Trainium2 Bass kernel: 3-layer GRU char decoder with greedy (argmax) rollout.

Data-parallel over the batch: 4096 rows -> 8 NeuronCores x 512 rows.
Weights are replicated per core and stay resident in SBUF; the whole
max_len-1 step recurrence runs on-device with no HBM traffic in the loop.

All matmuls run in native fp32 (PE 4 cycles/row) — the argmax feedback
loop is extremely sensitive to logit noise (bf16/f32r flip thousands of
rows), so precision dominates the design.

Layouts (per core, B=512):
  hT[l]     : SBUF [128, 4, B]   hidden state, d_h on (partition x ktile)
  weights   : W.T tiles [128, 1536] so matmul(psum, lhsT=W.T_tile, rhs=hT)
              computes gate pre-activations in [gate_dim, batch] layout
  logits    : [128 batch, 64 vocab] per batch tile -> argmax along free dim
  next x_emb: one-hot (exact) -> PE transpose -> matmul with emb table
"""

import numpy as np

PAD, SOS, EOS = 0, 1, 2
N, D_Z, D_H, V, D_E = 4096, 128, 512, 64, 128
NCORES = 8
B = N // NCORES  # 512 rows per core

_cache = {}


def _build(max_len: int, unroll: bool = False):
    import concourse.bass as bass
    import concourse.mybir as mybir
    import concourse.tile as tile
    from concourse import bacc

    f32 = mybir.dt.float32
    F = mybir.ActivationFunctionType
    Op = mybir.AluOpType
    AX = mybir.AxisListType

    nc = bacc.Bacc("TRN2", target_bir_lowering=False)

    # ---- DRAM I/O ----
    zT_d = nc.dram_tensor("zT", [D_Z, B], f32, kind="ExternalInput")
    z2hT_d = nc.dram_tensor("z2hT", [D_Z, D_H], f32, kind="ExternalInput")
    wiT_d = [
        nc.dram_tensor("wi0T", [D_E, 3 * D_H], f32, kind="ExternalInput"),
        nc.dram_tensor("wi1T", [D_H, 3 * D_H], f32, kind="ExternalInput"),
        nc.dram_tensor("wi2T", [D_H, 3 * D_H], f32, kind="ExternalInput"),
    ]
    whT_d = [
        nc.dram_tensor(f"wh{l}T", [D_H, 3 * D_H], f32, kind="ExternalInput")
        for l in range(3)
    ]
    h2vT_d = nc.dram_tensor("h2vT", [D_H, V], f32, kind="ExternalInput")
    emb_d = nc.dram_tensor("emb", [V, D_E], f32, kind="ExternalInput")
    x0embT_d = nc.dram_tensor("x0embT", [D_E, B], f32, kind="ExternalInput")
    b_rz_d = nc.dram_tensor("b_rz", [128, 24], f32, kind="ExternalInput")
    b_in_d = nc.dram_tensor("b_in", [128, 12], f32, kind="ExternalInput")
    b_hn_d = nc.dram_tensor("b_hn", [128, 12], f32, kind="ExternalInput")
    h0b_d = nc.dram_tensor("h0b", [128, 4], f32, kind="ExternalInput")
    h2vb_d = nc.dram_tensor("h2vb", [128, V], f32, kind="ExternalInput")
    iota_d = nc.dram_tensor("iota", [128, V], f32, kind="ExternalInput")
    iotam_d = nc.dram_tensor("iotam", [128, V], f32, kind="ExternalInput")
    ident_d = nc.dram_tensor("ident", [128, 128], f32, kind="ExternalInput")
    X_d = nc.dram_tensor("X", [B, max_len], f32, kind="ExternalOutput")
    seq_d = nc.dram_tensor("seq", [B], f32, kind="ExternalOutput")

    with tile.TileContext(nc) as tc:
        with (
            tc.tile_pool(name="pers", bufs=1) as pers,
            tc.tile_pool(name="wk", bufs=2) as wk,
            tc.tile_pool(name="wk64", bufs=2) as wk64,
            tc.tile_pool(name="wkS", bufs=2) as wkS,
            tc.tile_pool(name="pg", bufs=1, space="PSUM") as pg,
            tc.tile_pool(name="ps", bufs=1, space="PSUM") as ps,
            tc.tile_pool(name="ps2", bufs=2, space="PSUM") as ps2,
        ):
            # ---- persistent SBUF ----
            wi_sb = []
            for l in range(3):
                if l == 0:
                    t = pers.tile([D_E, 3 * D_H], f32, tag="wi0")
                    nc.sync.dma_start(t[:], wiT_d[0][:])
                    wi_sb.append([t])
                else:
                    ts_ = []
                    for k in range(4):
                        t = pers.tile([128, 3 * D_H], f32, tag=f"wi{l}k{k}")
                        nc.sync.dma_start(
                            t[:], wiT_d[l][128 * k : 128 * (k + 1), :])
                        ts_.append(t)
                    wi_sb.append(ts_)
            wh_sb = []
            for l in range(3):
                ts_ = []
                for k in range(4):
                    t = pers.tile([128, 3 * D_H], f32, tag=f"wh{l}k{k}")
                    nc.sync.dma_start(t[:], whT_d[l][128 * k : 128 * (k + 1), :])
                    ts_.append(t)
                wh_sb.append(ts_)
            h2v_sb = []
            for k in range(4):
                t = pers.tile([128, V], f32, tag=f"h2vk{k}")
                nc.sync.dma_start(t[:], h2vT_d[128 * k : 128 * (k + 1), :])
                h2v_sb.append(t)
            emb_sb = pers.tile([V, D_E], f32, tag="emb")
            nc.sync.dma_start(emb_sb[:], emb_d[:])
            xemb_sb = pers.tile([D_E, B], f32, tag="xemb")
            nc.sync.dma_start(xemb_sb[:], x0embT_d[:])
            b_rz_sb = pers.tile([128, 24], f32, tag="b_rz")
            nc.sync.dma_start(b_rz_sb[:], b_rz_d[:])
            b_in_sb = pers.tile([128, 12], f32, tag="b_in")
            nc.sync.dma_start(b_in_sb[:], b_in_d[:])
            b_hn_sb = pers.tile([128, 12], f32, tag="b_hn")
            nc.sync.dma_start(b_hn_sb[:], b_hn_d[:])
            h0b_sb = pers.tile([128, 4], f32, tag="h0b")
            nc.sync.dma_start(h0b_sb[:], h0b_d[:])
            h2vb_sb = pers.tile([128, V], f32, tag="h2vb")
            nc.sync.dma_start(h2vb_sb[:], h2vb_d[:])
            iota_sb = pers.tile([128, V], f32, tag="iota")
            nc.sync.dma_start(iota_sb[:], iota_d[:])
            iotam_sb = pers.tile([128, V], f32, tag="iotam")
            nc.sync.dma_start(iotam_sb[:], iotam_d[:])
            ident_sb = pers.tile([128, 128], f32, tag="ident")
            nc.sync.dma_start(ident_sb[:], ident_d[:])

            h_sb = [pers.tile([128, 4, B], f32, tag=f"h{l}", name=f"h{l}")
                    for l in range(3)]
            hn_sb = [pers.tile([128, 4, B], f32, tag=f"hn{l}", name=f"hn{l}")
                     for l in range(3)]
            ohT_sb = pers.tile([V, B], f32, tag="ohT")
            Xf = pers.tile([128, max_len * 4], f32, tag="Xf")
            seq_sb = pers.tile([128, 4], f32, tag="seq")
            eos_sb = pers.tile([128, 4], f32, tag="eos")
            idx_sb = pers.tile([128, 4], f32, tag="idx")
            cnt_sb = pers.tile([128, 1], f32, tag="cnt")

            nc.vector.memset(Xf[:], 0.0)
            nc.vector.memset(seq_sb[:], float(max_len))
            nc.vector.memset(eos_sb[:], 0.0)
            nc.vector.memset(cnt_sb[:], 2.0)

            # ---- h0 = Z @ z2h_w.T + b, replicated to all 3 layers ----
            zt_t = wk.tile([128, B], f32, tag="r_sb")
            nc.sync.dma_start(zt_t[:], zT_d[:])
            z2h_t = wk.tile([128, B], f32, tag="z_sb")
            nc.sync.dma_start(z2h_t[:], z2hT_d[:])
            for m in range(4):
                ph = pg.tile([128, B], f32, tag="p_r")
                nc.tensor.matmul(
                    ph[:], z2h_t[:, 128 * m : 128 * (m + 1)], zt_t[:],
                    start=True, stop=True)
                nc.scalar.activation(
                    h_sb[0][:, m, :], ph[:], F.Identity,
                    bias=h0b_sb[:, m : m + 1])
                nc.vector.tensor_copy(h_sb[1][:, m, :], h_sb[0][:, m, :])
                nc.vector.tensor_copy(h_sb[2][:, m, :], h_sb[0][:, m, :])

            # ---- one decode step ----
            def step(xcol):
                for l in range(3):
                    if l == 0:
                        x_tiles = [xemb_sb[:]]
                    else:
                        x_tiles = [hn_sb[l - 1][:, k, :] for k in range(4)]
                    nk = len(x_tiles)
                    wi, wh, h = wi_sb[l], wh_sb[l], h_sb[l]
                    for m in range(4):
                        pr = pg.tile([128, B], f32, tag="p_r")
                        pz = pg.tile([128, B], f32, tag="p_z")
                        pi = pg.tile([128, B], f32, tag="p_in")
                        phn = pg.tile([128, B], f32, tag="p_hn")
                        for p, base in ((pr, 0), (pz, D_H)):
                            col = base + 128 * m
                            for k in range(nk):
                                nc.tensor.matmul(
                                    p[:], wi[k][:, col : col + 128], x_tiles[k],
                                    start=(k == 0), stop=False)
                            for k in range(4):
                                nc.tensor.matmul(
                                    p[:], wh[k][:, col : col + 128], h[:, k, :],
                                    start=False, stop=(k == 3))
                        col = 2 * D_H + 128 * m
                        for k in range(nk):
                            nc.tensor.matmul(
                                pi[:], wi[k][:, col : col + 128], x_tiles[k],
                                start=(k == 0), stop=(k == nk - 1))
                        for k in range(4):
                            nc.tensor.matmul(
                                phn[:], wh[k][:, col : col + 128], h[:, k, :],
                                start=(k == 0), stop=(k == 3))
                        r = wk.tile([128, B], f32, tag="r_sb")
                        nc.scalar.activation(
                            r[:], pr[:], F.Sigmoid,
                            bias=b_rz_sb[:, 8 * l + m : 8 * l + m + 1])
                        z = wk.tile([128, B], f32, tag="z_sb")
                        nc.scalar.activation(
                            z[:], pz[:], F.Sigmoid,
                            bias=b_rz_sb[:, 8 * l + 4 + m : 8 * l + 5 + m])
                        t1 = wk.tile([128, B], f32, tag="t1")
                        nc.vector.scalar_tensor_tensor(
                            t1[:], phn[:], b_hn_sb[:, 4 * l + m : 4 * l + m + 1],
                            r[:], op0=Op.add, op1=Op.mult)
                        nc.vector.tensor_tensor(t1[:], t1[:], pi[:], Op.add)
                        nsb = wk.tile([128, B], f32, tag="n_sb")
                        nc.scalar.activation(
                            nsb[:], t1[:], F.Tanh,
                            bias=b_in_sb[:, 4 * l + m : 4 * l + m + 1])
                        dd = wk.tile([128, B], f32, tag="d_sb")
                        nc.vector.tensor_tensor(dd[:], h[:, m, :], nsb[:], Op.subtract)
                        nc.vector.tensor_tensor(dd[:], z[:], dd[:], Op.mult)
                        nc.vector.tensor_tensor(hn_sb[l][:, m, :], nsb[:], dd[:], Op.add)
                    for m in range(4):
                        nc.vector.tensor_copy(h[:, m, :], hn_sb[l][:, m, :])

                # logits -> argmax (first-index ties, matching jnp.argmax)
                h3 = hn_sb[2]
                for b in range(4):
                    pl = ps2.tile([128, V], f32, tag="p_log")
                    for k in range(4):
                        nc.tensor.matmul(
                            pl[:], h3[:, k, 128 * b : 128 * (b + 1)], h2v_sb[k][:],
                            start=(k == 0), stop=(k == 3))
                    lg = wk64.tile([128, V], f32, tag="lg")
                    nc.vector.tensor_tensor(lg[:], pl[:], h2vb_sb[:], Op.add)
                    mx = wkS.tile([128, 1], f32, tag="mx")
                    nc.vector.reduce_max(mx[:], lg[:], axis=AX.X)
                    msk = wk64.tile([128, V], f32, tag="msk")
                    nc.vector.tensor_scalar(
                        msk[:], lg[:], mx[:, 0:1], None, op0=Op.is_equal)
                    nc.vector.tensor_tensor(msk[:], msk[:], iotam_sb[:], Op.mult)
                    mn = wkS.tile([128, 1], f32, tag="mn")
                    nc.vector.tensor_reduce(mn[:], msk[:], axis=AX.X, op=Op.min)
                    nc.vector.tensor_scalar(
                        idx_sb[:, b : b + 1], mn[:], 128.0, None, op0=Op.add)

                # token bookkeeping (all exact small-int fp32 arithmetic)
                ise = wkS.tile([128, 4], f32, tag="ise")
                nc.vector.tensor_scalar(
                    ise[:], idx_sb[:], float(EOS), None, op0=Op.is_equal)
                tk = wkS.tile([128, 4], f32, tag="tk")
                nc.vector.tensor_tensor(tk[:], idx_sb[:], eos_sb[:], Op.mult)
                nc.vector.tensor_tensor(tk[:], idx_sb[:], tk[:], Op.subtract)
                nc.vector.tensor_copy(xcol, tk[:])
                nw = wkS.tile([128, 4], f32, tag="nw")
                nc.vector.tensor_tensor(nw[:], ise[:], eos_sb[:], Op.mult)
                nc.vector.tensor_tensor(nw[:], ise[:], nw[:], Op.subtract)
                sd = wkS.tile([128, 4], f32, tag="sd")
                nc.vector.tensor_scalar(
                    sd[:], seq_sb[:], cnt_sb[:, 0:1], None, op0=Op.subtract)
                nc.vector.tensor_tensor(sd[:], sd[:], nw[:], Op.mult)
                nc.vector.tensor_tensor(seq_sb[:], seq_sb[:], sd[:], Op.subtract)
                nc.vector.tensor_tensor(eos_sb[:], eos_sb[:], nw[:], Op.add)
                nc.vector.tensor_scalar(
                    cnt_sb[:], cnt_sb[:], 1.0, None, op0=Op.add)

                # next x_emb via exact one-hot matmul
                for b in range(4):
                    oh = wk64.tile([128, V], f32, tag="oh")
                    nc.vector.tensor_scalar(
                        oh[:], iota_sb[:], idx_sb[:, b : b + 1], None,
                        op0=Op.is_equal)
                    pt = ps.tile([V, 128], f32, tag="p_tr")
                    nc.tensor.transpose(pt[:], oh[:], ident_sb[:])
                    nc.vector.tensor_copy(ohT_sb[:, 128 * b : 128 * (b + 1)], pt[:])
                pe_ = ps.tile([D_E, B], f32, tag="p_emb")
                nc.tensor.matmul(pe_[:], emb_sb[:], ohT_sb[:], start=True, stop=True)
                nc.vector.tensor_copy(xemb_sb[:], pe_[:])

            if unroll:
                for t in range(1, max_len):
                    step(Xf[:, 4 * t : 4 * t + 4])
            else:
                import concourse.mybir as mybir_
                with tc.For_i(4, 4 * max_len, 4,
                              hint_engines=(mybir.EngineType.PE,)) as iv4:
                    step(Xf[:, bass.ds(iv4, 4)])

            # ---- writeback ----
            Xv = Xf[:].rearrange("p (t b) -> p t b", b=4)
            for b in range(4):
                nc.sync.dma_start(X_d[128 * b : 128 * (b + 1), :], Xv[:, :, b])
                nc.sync.dma_start(seq_d[128 * b : 128 * (b + 1)], seq_sb[:, b : b + 1])

    nc.compile()
    return nc


def _get(max_len: int, unroll: bool = False):
    key = (max_len, unroll)
    if key not in _cache:
        _cache[key] = _build(max_len, unroll)
    return _cache[key]


def kernel(**inputs):
    from concourse.bass_utils import run_bass_kernel_spmd

    max_len = int(np.asarray(inputs["max_len"]))
    f = np.float32
    a = lambda x: np.ascontiguousarray(np.asarray(x), dtype=f)

    Z = a(inputs["Z"])
    emb = a(inputs["emb"])
    shared = {
        "z2hT": a(np.asarray(inputs["z2h_w"]).T),
        "wi0T": a(np.asarray(inputs["W_ih0"]).T),
        "wi1T": a(np.asarray(inputs["W_ih1"]).T),
        "wi2T": a(np.asarray(inputs["W_ih2"]).T),
        "wh0T": a(np.asarray(inputs["W_hh0"]).T),
        "wh1T": a(np.asarray(inputs["W_hh1"]).T),
        "wh2T": a(np.asarray(inputs["W_hh2"]).T),
        "h2vT": a(np.asarray(inputs["h2v_w"]).T),
        "emb": emb,
        "x0embT": np.ascontiguousarray(np.tile(emb[SOS][:, None], (1, B))),
        "h0b": a(np.asarray(inputs["z2h_b"]).reshape(4, 128).T),
        "h2vb": np.ascontiguousarray(np.tile(a(inputs["h2v_b"])[None, :], (128, 1))),
        "iota": np.ascontiguousarray(np.tile(np.arange(V, dtype=f)[None, :], (128, 1))),
        "iotam": np.ascontiguousarray(
            np.tile(np.arange(V, dtype=f)[None, :] - 128.0, (128, 1)).astype(f)),
        "ident": np.eye(128, dtype=f),
    }
    for l in range(3):
        bi = a(inputs[f"b_ih{l}"])
        bh = a(inputs[f"b_hh{l}"])
        shared[f"_brz{l}"] = (bi[: 2 * D_H] + bh[: 2 * D_H]).reshape(8, 128).T
        shared[f"_bin{l}"] = bi[2 * D_H :].reshape(4, 128).T
        shared[f"_bhn{l}"] = bh[2 * D_H :].reshape(4, 128).T
    shared["b_rz"] = np.ascontiguousarray(
        np.concatenate([shared.pop(f"_brz{l}") for l in range(3)], axis=1))
    shared["b_in"] = np.ascontiguousarray(
        np.concatenate([shared.pop(f"_bin{l}") for l in range(3)], axis=1))
    shared["b_hn"] = np.ascontiguousarray(
        np.concatenate([shared.pop(f"_bhn{l}") for l in range(3)], axis=1))

    in_maps = []
    for c in range(NCORES):
        m = dict(shared)
        m["zT"] = np.ascontiguousarray(Z[c * B : (c + 1) * B].T)
        in_maps.append(m)

    nc = _get(max_len)
    res = run_bass_kernel_spmd(nc, in_maps, core_ids=list(range(NCORES)))
    X = np.concatenate([r["X"] for r in res.results], axis=0)
    seq = np.concatenate([r["seq"] for r in res.results], axis=0)
    Xi = np.rint(X).astype(np.int32)
    Xi[:, 0] = SOS
    return Xi, np.rint(seq).astype(np.int32)
